# revision 31
# baseline (speedup 1.0000x reference)
"""Causal multi-head attention (B=4, H=16, S=2048, D=64) on 8 TRN2 NeuronCores.

Sharding: 64 (batch, head) pairs, 8 per core, processed as 4 "duos" (X, Y).
q/k are host-pre-transposed to d-major and duo-stacked: X's 64 d-rows on
SBUF partitions 0-63, Y's on 64-127.

Per-duo algorithm (flash-attention, transposed-score layout), per q-tile
("unit", 512 q cols), over k-tiles t in the causal lower triangle:

S^T stage - ROW-TILED matmul pairs: X's S^T at tile_position (0,0) using PE
rows 0-63, Y's at (64,0) using rows 64-127. The two contraction-64 matmuls
run concurrently in the PE array, writing one [128, 2, 512] two-bank PSUM
super-tile; full-array activity also keeps the HAM clock monitor at K=8/8
(2.4 GHz) - half-array streams measurably never leave the 1.2 GHz cold
clock (the v1 kernel's core defect: every matmul ran at (219+N)/1.2 ns).

exp - one wide instruction per k-tile covering both streams ((N+352)/1.2 ns
on ScalarE: batching amortizes the 352-cycle overhead). ScalarE alone
(~46us/duo) would cap the kernel below the PE's ~30us/duo, so k-tiles
alternate between ScalarE exact exp and a VectorE Schraudolph bit-trick
exp: int16(round(x*128*log2e*scale + (16256 - 128*0.043677))) bitcast to
bf16 ~= exp(x*scale) within +-3 percent (measured end-to-end rel-err 0.010
at a 0.02 gate; all-ScalarE would be 0.003). EXP_PATTERN must alternate
strictly - two adjacent "s" slots overload ScalarE within one pipeline
group and cost ~4-30us. Causal masks: a single GPSIMD affine_select over
both streams' first 128 columns of each diagonal tile; diagonal k-tiles
are emitted FIRST in each unit so their longer exp->mask chain never lands
on the end-of-unit PV flush (PV summation order is commutative).

PV stage - acc[65, 512] += V'[128, 65]^T @ P[128, w] per stream; V' carries
a ones column so acc row 64 accumulates the softmax denominator for free.
PV-pair emission lags the S-pairs by one GR=4 group so the in-order PE
queue always holds work that is independent of pending exps (3 ps
super-tile buffers cover the lag; the tail transpose scratch is carved from
a ps slot's first bank to stay within 8 PSUM banks total).

Unit tails (deferred into the next unit's first groups, X/Y staggered):
evict acc -> SBUF bf16 (alternating ScalarE/VectorE), transpose via 4
identity matmuls into one PSUM bank [128, 4, 65], one strided reciprocal of
the 4 denominator columns, one broadcast tensor_tensor normalize mul, one
DMA of [512, 64] fp32 to DRAM in natural [q, d] layout.

The first duo's q/k DMAs land the j=0 columns first so the PE starts ~2us
earlier; per-engine-queue DMA spreading measurably backfires (a dma_start
occupies its issuing engine for the whole transfer).
"""

import math

import numpy as np
import ml_dtypes

import concourse.bass as bass
import concourse.bacc as bacc
import concourse.tile as tile
import concourse.mybir as mybir
from concourse import bass_utils
from concourse.masks import make_identity

B, H, S, D = 4, 16, 2048, 64
N_CORES = 8
PAIRS = (B * H) // N_CORES  # 8 heads per core
DUOS = PAIRS // 2           # 4 lockstep duos per core
QT = 512                    # q-tile width
KT = 128                    # k-tile rows
NQT = S // QT               # 4 q-tiles per head
GR = 4                      # k-tiles per pipeline group
SCALE = 1.0 / math.sqrt(D)
A_SCH = (128.0 / math.log(2.0)) * SCALE       # Schraudolph slope (scale folded)
B_SCH = 16256.0 - 128.0 * 0.043677            # Schraudolph offset (bf16 bias)
EXP_PATTERN = ("s", "v", "s", "v", "s", "v", "s", "v", "s")  # 5/9 ScalarE exact
BF16 = ml_dtypes.bfloat16

_COMPILED = {}


def build_nc():
    nc = bacc.Bacc(
        "TRN2",
        target_bir_lowering=False,
        debug=False,
        enable_asserts=True,
        num_devices=N_CORES,
    )
    f32 = mybir.dt.float32
    bf16 = mybir.dt.bfloat16
    i16 = mybir.dt.int16

    qt_d = nc.dram_tensor("qt", [DUOS * 2 * D, S], bf16, kind="ExternalInput").ap()
    kt_d = nc.dram_tensor("kt", [DUOS * 2 * D, S], bf16, kind="ExternalInput").ap()
    v_d = nc.dram_tensor("v", [PAIRS * S, D], bf16, kind="ExternalInput").ap()
    out_d = nc.dram_tensor("out", [PAIRS * S, D], f32, kind="ExternalOutput").ap()

    with tile.TileContext(nc) as tc:
        with (
            tc.tile_pool(name="consts", bufs=1) as consts,
            tc.tile_pool(name="qk", bufs=2) as qk_pool,
            tc.tile_pool(name="vp", bufs=2) as v_pool,
            tc.tile_pool(name="pp", bufs=8) as p_pool,
            tc.tile_pool(name="op", bufs=3) as o_pool,
            tc.tile_pool(name="fp", bufs=3) as f_pool,
            tc.tile_pool(name="rp", bufs=3) as r_pool,
            tc.tile_pool(name="big", bufs=3, space="PSUM") as big_pool,
            tc.tile_pool(name="acc", bufs=2, space="PSUM") as acc_pool,
        ):
            ident = consts.tile([D + 1, D + 1], bf16)
            make_identity(nc, ident)
            st = {"exp": 0, "tail": 0}

            def load_duo(dd):
                qsb = qk_pool.tile([2 * D, S], bf16, tag="qsb", name=f"q{dd}")
                ksb = qk_pool.tile([2 * D, S], bf16, tag="ksb", name=f"k{dd}")
                r0, r1 = dd * 128, (dd + 1) * 128
                if dd == 0:
                    # first duo: land the j=0 columns first so the PE starts
                    nc.sync.dma_start(out=qsb[:, 0:QT], in_=qt_d[r0:r1, 0:QT])
                    nc.sync.dma_start(out=ksb[:, 0:QT], in_=kt_d[r0:r1, 0:QT])
                    nc.sync.dma_start(out=qsb[:, QT:], in_=qt_d[r0:r1, QT:])
                    nc.sync.dma_start(out=ksb[:, QT:], in_=kt_d[r0:r1, QT:])
                else:
                    nc.sync.dma_start(out=qsb, in_=qt_d[r0:r1, :])
                    nc.sync.dma_start(out=ksb, in_=kt_d[r0:r1, :])
                vs = []
                for s_ in range(2):
                    h = 2 * dd + s_
                    vt = v_pool.tile([KT, S // KT, D + 1], bf16, tag=f"v{s_}",
                                     name=f"v{dd}_{s_}")
                    nc.gpsimd.memset(vt[:, :, D:D + 1], 1.0)
                    nc.sync.dma_start(
                        out=vt[:, :, 0:D],
                        in_=v_d[h * S:(h + 1) * S, :].rearrange(
                            "(t kp) d -> kp t d", kp=KT),
                    )
                    vs.append(vt)
                return qsb, ksb, vs

            def emit_s(sb, j, t):
                qsb, ksb, _ = sb
                off = max(0, KT * t - QT * j)
                w = QT - off
                q0 = QT * j + off
                ps = big_pool.tile([KT, 2, QT], f32, tag="ps", name="ps")
                nc.tensor.matmul(
                    ps[:, 0, 0:w],
                    lhsT=ksb[0:D, KT * t:KT * (t + 1)],
                    rhs=qsb[0:D, q0:QT * (j + 1)],
                    start=True, stop=True, tile_position=(0, 0),
                )
                nc.tensor.matmul(
                    ps[:, 1, 0:w],
                    lhsT=ksb[D:2 * D, KT * t:KT * (t + 1)],
                    rhs=qsb[D:2 * D, q0:QT * (j + 1)],
                    start=True, stop=True, tile_position=(64, 0),
                )
                p3 = p_pool.tile([KT, 2, QT], bf16, tag="p3", name="p3")
                eng = EXP_PATTERN[st["exp"] % len(EXP_PATTERN)]
                st["exp"] += 1
                if eng == "s":
                    nc.scalar.activation(
                        out=p3[:, :, 0:w], in_=ps[:, :, 0:w],
                        func=mybir.ActivationFunctionType.Exp, scale=SCALE,
                    )
                else:
                    nc.vector.tensor_scalar(
                        out=p3[:, :, 0:w].bitcast(i16), in0=ps[:, :, 0:w],
                        scalar1=A_SCH, scalar2=B_SCH,
                        op0=mybir.AluOpType.mult, op1=mybir.AluOpType.add,
                    )
                if t >= (QT // KT) * j:  # diagonal tile: zero q_rel < k_rel
                    nc.gpsimd.affine_select(
                        out=p3[:, :, 0:KT], in_=p3[:, :, 0:KT],
                        compare_op=mybir.AluOpType.is_ge,
                        fill=0.0, base=0,
                        pattern=[[0, 2], [1, KT]], channel_multiplier=-1,
                    )
                return p3, off, w

            def emit_pv(sb, accs, t, p3off, first, last):
                p3, off, w = p3off
                for s_ in range(2):
                    nc.tensor.matmul(
                        accs[s_][:, off:QT],
                        lhsT=sb[2][s_][:, t, :],
                        rhs=p3[:, s_, 0:w],
                        start=first, stop=last,
                    )

            def emit_tail(h, j, acc):
                osb = o_pool.tile([D + 1, QT], bf16, tag="osb", name="osb")
                if st["tail"] % 2 == 0:
                    nc.vector.tensor_copy(osb, acc)
                else:
                    nc.scalar.copy(out=osb, in_=acc)
                st["tail"] += 1
                # transpose scratch carved from a ps slot's first bank
                pst = big_pool.tile([KT, 2, QT], f32, tag="ps", name="tpslot")
                tp = pst[:, 0, 0:(QT // KT) * (D + 1)].rearrange(
                    "p (b c) -> p b c", b=QT // KT)
                for b_ in range(QT // KT):
                    nc.tensor.matmul(
                        tp[:, b_, :],
                        lhsT=osb[:, KT * b_:KT * (b_ + 1)],
                        rhs=ident, start=True, stop=True,
                    )
                rinv = r_pool.tile([KT, QT // KT], f32, tag="ri", name="rinv")
                nc.vector.reciprocal(rinv, tp[:, :, D])
                fsb = f_pool.tile([KT, QT // KT, D], f32, tag="f", name="fsb")
                # one mul for all 4 blocks: rinv broadcast along d (stride 0)
                nc.vector.tensor_tensor(
                    out=fsb, in0=tp[:, :, 0:D],
                    in1=rinv.broadcast_to([KT, QT // KT, D]),
                    op=mybir.AluOpType.mult,
                )
                row0 = h * S + QT * j
                nc.sync.dma_start(
                    out=out_d[row0:row0 + QT, :].rearrange(
                        "(b p) d -> p b d", p=KT),
                    in_=fsb,
                )

            sbs = load_duo(0)
            sbs_next = None
            pending_tails = []
            for dd in range(DUOS):
                sb = sbs
                for j in range(NQT):
                    nkt = (QT // KT) * (j + 1)
                    ngr = nkt // GR
                    accx = acc_pool.tile([D + 1, QT], f32, tag="acc",
                                         name="accx")
                    accy = acc_pool.tile([D + 1, QT], f32, tag="acc",
                                         name="accy")
                    accs = (accx, accy)
                    # diagonal k-tiles first: their exp->mask chain is the
                    # longest, so keep them away from the end-of-unit flush
                    # where no S-work remains to hide the wait. PV summation
                    # order over k is commutative; start/stop flags follow
                    # emission order.
                    t_order = list(range(4 * j, nkt)) + list(range(4 * j))
                    npop = 0
                    pend = []

                    def pop_pv():
                        nonlocal npop
                        tt, p3off = pend.pop(0)
                        emit_pv(sb, accs, tt, p3off,
                                first=(npop == 0), last=(npop == nkt - 1))
                        npop += 1

                    for g in range(ngr):
                        for half in range(GR):
                            t = t_order[GR * g + half]
                            pend.append((t, emit_s(sb, j, t)))
                        # stagger the two tails across groups so their
                        # ScalarE/DVE work doesn't pile onto one exp window
                        if pending_tails and g < 2:
                            emit_tail(*pending_tails.pop(0))
                            if ngr == 1:
                                while pending_tails:
                                    emit_tail(*pending_tails.pop(0))
                        if g == 0 and j == 1 and dd + 1 < DUOS:
                            sbs_next = load_duo(dd + 1)
                        while len(pend) > GR:
                            pop_pv()
                    while pend:
                        pop_pv()
                    pending_tails = [(2 * dd, j, accx), (2 * dd + 1, j, accy)]
                sbs = sbs_next
            for args in pending_tails:
                emit_tail(*args)

    nc.compile()
    return nc


def _get_nc():
    if "nc" not in _COMPILED:
        _COMPILED["nc"] = build_nc()
    return _COMPILED["nc"]


def make_in_maps(q, k, v):
    q = np.asarray(q, dtype=np.float32).reshape(B * H, S, D)
    k = np.asarray(k, dtype=np.float32).reshape(B * H, S, D)
    v = np.asarray(v, dtype=np.float32).reshape(B * H, S, D)
    in_maps = []
    for c in range(N_CORES):
        sl = slice(c * PAIRS, (c + 1) * PAIRS)
        # duo-stacked d-major [DUOS*128, S]: duo dd rows 0-63 = head 2dd,
        # rows 64-127 = head 2dd+1
        qt = np.ascontiguousarray(q[sl].transpose(0, 2, 1)).reshape(
            DUOS * 2 * D, S)
        kt = np.ascontiguousarray(k[sl].transpose(0, 2, 1)).reshape(
            DUOS * 2 * D, S)
        in_maps.append({
            "qt": qt.astype(BF16),
            "kt": kt.astype(BF16),
            "v": np.ascontiguousarray(v[sl]).reshape(PAIRS * S, D).astype(BF16),
        })
    return in_maps


def assemble(results):
    out = np.empty((B * H, S, D), dtype=np.float32)
    for c in range(N_CORES):
        out[c * PAIRS:(c + 1) * PAIRS] = results[c]["out"].reshape(PAIRS, S, D)
    return np.ascontiguousarray(
        out.reshape(B, H, S, D).transpose(0, 2, 1, 3).reshape(B, S, H * D))


def kernel(q, k, v):
    nc = _get_nc()
    res = bass_utils.run_bass_kernel_spmd(
        nc, make_in_maps(q, k, v), core_ids=list(range(N_CORES)))
    return assemble(res.results)


# revision 32
# speedup vs baseline: 1.0430x; 1.0430x over previous
"""Causal multi-head attention (B=4, H=16, S=2048, D=64) on 8 TRN2 NeuronCores.

Sharding: 64 (batch, head) pairs, 8 per core, processed as 4 "duos" (X, Y).
q/k are host-pre-transposed to d-major and duo-stacked: X's 64 d-rows on
SBUF partitions 0-63, Y's on 64-127.

Per-duo algorithm (flash-attention, transposed-score layout), per q-tile
("unit", 512 q cols), over k-tiles t in the causal lower triangle:

S^T stage - ROW-TILED matmul pairs: X's S^T at tile_position (0,0) using PE
rows 0-63, Y's at (64,0) using rows 64-127. The two contraction-64 matmuls
run concurrently in the PE array, writing one [128, 2, 512] two-bank PSUM
super-tile; full-array activity also keeps the HAM clock monitor at K=8/8
(2.4 GHz) - half-array streams measurably never leave the 1.2 GHz cold
clock (the v1 kernel's core defect: every matmul ran at (219+N)/1.2 ns).

exp - one wide instruction per k-tile covering both streams ((N+352)/1.2 ns
on ScalarE: batching amortizes the 352-cycle overhead). ScalarE alone
(~46us/duo) would cap the kernel below the PE's ~30us/duo, so k-tiles
alternate between ScalarE exact exp and a VectorE Schraudolph bit-trick
exp: int16(round(x*128*log2e*scale + (16256 - 128*0.043677))) bitcast to
bf16 ~= exp(x*scale) within +-3 percent (measured end-to-end rel-err 0.010
at a 0.02 gate; all-ScalarE would be 0.003). EXP_PATTERN must alternate
strictly - two adjacent "s" slots overload ScalarE within one pipeline
group and cost ~4-30us. Causal masks: a single GPSIMD affine_select over
both streams' first 128 columns of each diagonal tile; diagonal k-tiles
are emitted FIRST in each unit so their longer exp->mask chain never lands
on the end-of-unit PV flush (PV summation order is commutative).

PV stage - acc[65, 512] += V'[128, 65]^T @ P[128, w] per stream; V' carries
a ones column so acc row 64 accumulates the softmax denominator for free.
PV-pair emission lags the S-pairs by one GR=4 group so the in-order PE
queue always holds work that is independent of pending exps (3 ps
super-tile buffers cover the lag; the tail transpose scratch is carved from
a ps slot's first bank to stay within 8 PSUM banks total).

Unit tails (deferred into the next unit's first groups, X/Y staggered):
evict acc -> SBUF bf16 (alternating ScalarE/VectorE), transpose via 4
identity matmuls into one PSUM bank [128, 4, 65], one strided reciprocal of
the 4 denominator columns, one broadcast tensor_tensor normalize mul, one
DMA of [512, 64] fp32 to DRAM in natural [q, d] layout.

The first duo's q/k DMAs land the j=0 columns first so the PE starts ~2us
earlier; per-engine-queue DMA spreading measurably backfires (a dma_start
occupies its issuing engine for the whole transfer).
"""

import math

import numpy as np
import ml_dtypes

import concourse.bass as bass
import concourse.bacc as bacc
import concourse.tile as tile
import concourse.mybir as mybir
from concourse import bass_utils
from concourse.masks import make_identity

B, H, S, D = 4, 16, 2048, 64
N_CORES = 8
PAIRS = (B * H) // N_CORES  # 8 heads per core
DUOS = PAIRS // 2           # 4 lockstep duos per core
QT = 512                    # q-tile width
KT = 128                    # k-tile rows
NQT = S // QT               # 4 q-tiles per head
GR = 4                      # k-tiles per pipeline group
SCALE = 1.0 / math.sqrt(D)
A_SCH = (128.0 / math.log(2.0)) * SCALE       # Schraudolph slope (scale folded)
B_SCH = 16256.0 - 128.0 * 0.043677            # Schraudolph offset (bf16 bias)
EXP_PATTERN = ("s", "v", "s", "v", "s", "v", "s", "v", "s")  # 5/9 ScalarE exact
BF16 = ml_dtypes.bfloat16

_COMPILED = {}


def build_nc():
    nc = bacc.Bacc(
        "TRN2",
        target_bir_lowering=False,
        debug=False,
        enable_asserts=True,
        num_devices=N_CORES,
    )
    f32 = mybir.dt.float32
    bf16 = mybir.dt.bfloat16
    i16 = mybir.dt.int16

    qt_d = nc.dram_tensor("qt", [DUOS * 2 * D, S], bf16, kind="ExternalInput").ap()
    kt_d = nc.dram_tensor("kt", [DUOS * 2 * D, S], bf16, kind="ExternalInput").ap()
    v_d = nc.dram_tensor("v", [PAIRS * S, D], bf16, kind="ExternalInput").ap()
    out_d = nc.dram_tensor("out", [PAIRS * S, D], f32, kind="ExternalOutput").ap()

    with tile.TileContext(nc) as tc:
        with (
            tc.tile_pool(name="consts", bufs=1) as consts,
            tc.tile_pool(name="qk", bufs=2) as qk_pool,
            tc.tile_pool(name="vp", bufs=2) as v_pool,
            tc.tile_pool(name="pp", bufs=8) as p_pool,
            tc.tile_pool(name="op", bufs=3) as o_pool,
            tc.tile_pool(name="fp", bufs=3) as f_pool,
            tc.tile_pool(name="rp", bufs=3) as r_pool,
            tc.tile_pool(name="big", bufs=3, space="PSUM") as big_pool,
            tc.tile_pool(name="acc", bufs=2, space="PSUM") as acc_pool,
        ):
            ident = consts.tile([D + 1, D + 1], bf16)
            make_identity(nc, ident)
            st = {"exp": 0, "tail": 0}

            def load_duo(dd):
                qsb = qk_pool.tile([2 * D, S], bf16, tag="qsb", name=f"q{dd}")
                ksb = qk_pool.tile([2 * D, S], bf16, tag="ksb", name=f"k{dd}")
                r0, r1 = dd * 128, (dd + 1) * 128
                if dd == 0:
                    # first duo: land the j=0 columns first so the PE starts
                    nc.sync.dma_start(out=qsb[:, 0:QT], in_=qt_d[r0:r1, 0:QT])
                    nc.sync.dma_start(out=ksb[:, 0:QT], in_=kt_d[r0:r1, 0:QT])
                    nc.sync.dma_start(out=qsb[:, QT:], in_=qt_d[r0:r1, QT:])
                    nc.sync.dma_start(out=ksb[:, QT:], in_=kt_d[r0:r1, QT:])
                else:
                    nc.sync.dma_start(out=qsb, in_=qt_d[r0:r1, :])
                    nc.sync.dma_start(out=ksb, in_=kt_d[r0:r1, :])
                vs = []
                for s_ in range(2):
                    h = 2 * dd + s_
                    vt = v_pool.tile([KT, S // KT, D + 1], bf16, tag=f"v{s_}",
                                     name=f"v{dd}_{s_}")
                    nc.gpsimd.memset(vt[:, :, D:D + 1], 1.0)
                    nc.sync.dma_start(
                        out=vt[:, :, 0:D],
                        in_=v_d[h * S:(h + 1) * S, :].rearrange(
                            "(t kp) d -> kp t d", kp=KT),
                    )
                    vs.append(vt)
                return qsb, ksb, vs

            def emit_s(sb, j, t):
                qsb, ksb, _ = sb
                off = max(0, KT * t - QT * j)
                w = QT - off
                q0 = QT * j + off
                ps = big_pool.tile([KT, 2, QT], f32, tag="ps", name="ps")
                nc.tensor.matmul(
                    ps[:, 0, 0:w],
                    lhsT=ksb[0:D, KT * t:KT * (t + 1)],
                    rhs=qsb[0:D, q0:QT * (j + 1)],
                    start=True, stop=True, tile_position=(0, 0),
                )
                nc.tensor.matmul(
                    ps[:, 1, 0:w],
                    lhsT=ksb[D:2 * D, KT * t:KT * (t + 1)],
                    rhs=qsb[D:2 * D, q0:QT * (j + 1)],
                    start=True, stop=True, tile_position=(64, 0),
                )
                p3 = p_pool.tile([KT, 2, QT], bf16, tag="p3", name="p3")
                eng = EXP_PATTERN[st["exp"] % len(EXP_PATTERN)]
                st["exp"] += 1
                if eng == "s":
                    nc.scalar.activation(
                        out=p3[:, :, 0:w], in_=ps[:, :, 0:w],
                        func=mybir.ActivationFunctionType.Exp, scale=SCALE,
                    )
                else:
                    nc.vector.tensor_scalar(
                        out=p3[:, :, 0:w].bitcast(i16), in0=ps[:, :, 0:w],
                        scalar1=A_SCH, scalar2=B_SCH,
                        op0=mybir.AluOpType.mult, op1=mybir.AluOpType.add,
                    )
                if t >= (QT // KT) * j:  # diagonal tile: zero q_rel < k_rel
                    nc.gpsimd.affine_select(
                        out=p3[:, :, 0:KT], in_=p3[:, :, 0:KT],
                        compare_op=mybir.AluOpType.is_ge,
                        fill=0.0, base=0,
                        pattern=[[0, 2], [1, KT]], channel_multiplier=-1,
                    )
                return p3, off, w

            def emit_pv(sb, accs, t, p3off, first, last):
                p3, off, w = p3off
                for s_ in range(2):
                    nc.tensor.matmul(
                        accs[s_][:, off:QT],
                        lhsT=sb[2][s_][:, t, :],
                        rhs=p3[:, s_, 0:w],
                        start=first, stop=last,
                    )

            def emit_tail(h, j, acc):
                osb = o_pool.tile([D + 1, QT], bf16, tag="osb", name="osb")
                if st["tail"] % 2 == 0:
                    nc.vector.tensor_copy(osb, acc)
                else:
                    nc.scalar.copy(out=osb, in_=acc)
                st["tail"] += 1
                # transpose scratch carved from a ps slot's first bank
                pst = big_pool.tile([KT, 2, QT], f32, tag="ps", name="tpslot")
                tp = pst[:, 0, 0:(QT // KT) * (D + 1)].rearrange(
                    "p (b c) -> p b c", b=QT // KT)
                for b_ in range(QT // KT):
                    nc.tensor.matmul(
                        tp[:, b_, :],
                        lhsT=osb[:, KT * b_:KT * (b_ + 1)],
                        rhs=ident, start=True, stop=True,
                    )
                rinv = r_pool.tile([KT, QT // KT], f32, tag="ri", name="rinv")
                nc.vector.reciprocal(rinv, tp[:, :, D])
                fsb = f_pool.tile([KT, QT // KT, D], f32, tag="f", name="fsb")
                # one mul for all 4 blocks: rinv broadcast along d (stride 0)
                nc.vector.tensor_tensor(
                    out=fsb, in0=tp[:, :, 0:D],
                    in1=rinv.broadcast_to([KT, QT // KT, D]),
                    op=mybir.AluOpType.mult,
                )
                row0 = h * S + QT * j
                nc.sync.dma_start(
                    out=out_d[row0:row0 + QT, :].rearrange(
                        "(b p) d -> p b d", p=KT),
                    in_=fsb,
                )

            sbs = load_duo(0)
            sbs_next = None
            LAG = GR
            pend = []           # (sb, accs, nkt, t, p3off, uid)
            npop = {}           # uid -> PVs emitted so far
            pending_tails = []  # (uid, head, j, acc)

            def pop_pv():
                psb, paccs, pnkt, tt, p3off, uid = pend.pop(0)
                k = npop.get(uid, 0)
                emit_pv(psb, paccs, tt, p3off,
                        first=(k == 0), last=(k == pnkt - 1))
                npop[uid] = k + 1

            def emit_ready_tails():
                # a unit's tails may go once all its PVs are emitted; placed
                # at group boundaries so the transpose matmuls never land
                # mid-PV-run (which breaks MM chaining)
                while pending_tails and npop.get(pending_tails[0][0], 0) == \
                        4 * (pending_tails[0][2] + 1):
                    emit_tail(*pending_tails.pop(0)[1:])

            for dd in range(DUOS):
                sb = sbs
                for j in range(NQT):
                    uid = dd * NQT + j
                    nkt = (QT // KT) * (j + 1)
                    accx = acc_pool.tile([D + 1, QT], f32, tag="acc",
                                         name="accx")
                    accy = acc_pool.tile([D + 1, QT], f32, tag="acc",
                                         name="accy")
                    accs = (accx, accy)
                    # diagonal k-tiles first (longest exp->mask chain)
                    t_order = list(range(4 * j, nkt)) + list(range(4 * j))
                    for g in range(nkt // GR):
                        for half in range(GR):
                            t = t_order[GR * g + half]
                            pend.append((sb, accs, nkt, t,
                                         emit_s(sb, j, t), uid))
                        if g == 0 and j == 1 and dd + 1 < DUOS:
                            sbs_next = load_duo(dd + 1)
                        while len(pend) > LAG:
                            pop_pv()
                        emit_ready_tails()
                    pending_tails.append((uid, 2 * dd, j, accx))
                    pending_tails.append((uid, 2 * dd + 1, j, accy))
                sbs = sbs_next
            while pend:
                pop_pv()
            for args in pending_tails:
                emit_tail(*args[1:])

    nc.compile()
    return nc


def _get_nc():
    if "nc" not in _COMPILED:
        _COMPILED["nc"] = build_nc()
    return _COMPILED["nc"]


def make_in_maps(q, k, v):
    q = np.asarray(q, dtype=np.float32).reshape(B * H, S, D)
    k = np.asarray(k, dtype=np.float32).reshape(B * H, S, D)
    v = np.asarray(v, dtype=np.float32).reshape(B * H, S, D)
    in_maps = []
    for c in range(N_CORES):
        sl = slice(c * PAIRS, (c + 1) * PAIRS)
        # duo-stacked d-major [DUOS*128, S]: duo dd rows 0-63 = head 2dd,
        # rows 64-127 = head 2dd+1
        qt = np.ascontiguousarray(q[sl].transpose(0, 2, 1)).reshape(
            DUOS * 2 * D, S)
        kt = np.ascontiguousarray(k[sl].transpose(0, 2, 1)).reshape(
            DUOS * 2 * D, S)
        in_maps.append({
            "qt": qt.astype(BF16),
            "kt": kt.astype(BF16),
            "v": np.ascontiguousarray(v[sl]).reshape(PAIRS * S, D).astype(BF16),
        })
    return in_maps


def assemble(results):
    out = np.empty((B * H, S, D), dtype=np.float32)
    for c in range(N_CORES):
        out[c * PAIRS:(c + 1) * PAIRS] = results[c]["out"].reshape(PAIRS, S, D)
    return np.ascontiguousarray(
        out.reshape(B, H, S, D).transpose(0, 2, 1, 3).reshape(B, S, H * D))


def kernel(q, k, v):
    nc = _get_nc()
    res = bass_utils.run_bass_kernel_spmd(
        nc, make_in_maps(q, k, v), core_ids=list(range(N_CORES)))
    return assemble(res.results)


# revision 34
# speedup vs baseline: 1.0486x; 1.0054x over previous
"""Causal multi-head attention (B=4, H=16, S=2048, D=64) on 8 TRN2 NeuronCores.

Sharding: 64 (batch, head) pairs, 8 per core, processed as 4 "duos" (X, Y).
q/k are host-pre-transposed to d-major and duo-stacked: X's 64 d-rows on
SBUF partitions 0-63, Y's on 64-127.

Per-duo algorithm (flash-attention, transposed-score layout), per q-tile
("unit", 512 q cols), over k-tiles t in the causal lower triangle:

S^T stage - ROW-TILED matmul pairs: X's S^T at tile_position (0,0) using PE
rows 0-63, Y's at (64,0) using rows 64-127. The two contraction-64 matmuls
run concurrently in the PE array, writing one [128, 2, 512] two-bank PSUM
super-tile; full-array activity also keeps the HAM clock monitor at K=8/8
(2.4 GHz) - half-array streams measurably never leave the 1.2 GHz cold
clock (the v1 kernel's core defect: every matmul ran at (219+N)/1.2 ns).

exp - one wide instruction per k-tile covering both streams ((N+352)/1.2 ns
on ScalarE: batching amortizes the 352-cycle overhead). ScalarE alone
(~46us/duo) would cap the kernel below the PE's ~30us/duo, so k-tiles
alternate between ScalarE exact exp and a VectorE Schraudolph bit-trick
exp: int16(round(x*128*log2e*scale + (16256 - 128*0.043677))) bitcast to
bf16 ~= exp(x*scale) within +-3 percent (measured end-to-end rel-err 0.010
at a 0.02 gate; all-ScalarE would be 0.003). EXP_PATTERN must alternate
strictly - two adjacent "s" slots overload ScalarE within one pipeline
group and cost ~4-30us. Causal masks: a single GPSIMD affine_select over
both streams' first 128 columns of each diagonal tile; diagonal k-tiles
are emitted FIRST in each unit so their longer exp->mask chain never lands
on the end-of-unit PV flush (PV summation order is commutative).

PV stage - acc[65, 512] += V'[128, 65]^T @ P[128, w] per stream; V' carries
a ones column so acc row 64 accumulates the softmax denominator for free.
PV-pair emission lags the S-pairs by one GR=4 group so the in-order PE
queue always holds work that is independent of pending exps (3 ps
super-tile buffers cover the lag; the tail transpose scratch is carved from
a ps slot's first bank to stay within 8 PSUM banks total).

Unit tails (deferred into the next unit's first groups, X/Y staggered):
evict acc -> SBUF bf16 (alternating ScalarE/VectorE), transpose via 4
identity matmuls into one PSUM bank [128, 4, 65], one strided reciprocal of
the 4 denominator columns, one broadcast tensor_tensor normalize mul, one
DMA of [512, 64] fp32 to DRAM in natural [q, d] layout.

The first duo's q/k DMAs land the j=0 columns first so the PE starts ~2us
earlier; per-engine-queue DMA spreading measurably backfires (a dma_start
occupies its issuing engine for the whole transfer).
"""

import math

import numpy as np
import ml_dtypes

import concourse.bass as bass
import concourse.bacc as bacc
import concourse.tile as tile
import concourse.mybir as mybir
from concourse import bass_utils
from concourse.masks import make_identity

B, H, S, D = 4, 16, 2048, 64
N_CORES = 8
PAIRS = (B * H) // N_CORES  # 8 heads per core
DUOS = PAIRS // 2           # 4 lockstep duos per core
QT = 512                    # q-tile width
KT = 128                    # k-tile rows
NQT = S // QT               # 4 q-tiles per head
GR = 4                      # k-tiles per pipeline group
SCALE = 1.0 / math.sqrt(D)
A_SCH = (128.0 / math.log(2.0)) * SCALE       # Schraudolph slope (scale folded)
B_SCH = 16256.0 - 128.0 * 0.043677            # Schraudolph offset (bf16 bias)
EXP_PATTERN = ("s", "v", "s", "v", "s", "v", "s", "v", "s")  # 5/9 ScalarE exact
BF16 = ml_dtypes.bfloat16

_COMPILED = {}


def build_nc():
    nc = bacc.Bacc(
        "TRN2",
        target_bir_lowering=False,
        debug=False,
        enable_asserts=True,
        num_devices=N_CORES,
    )
    f32 = mybir.dt.float32
    bf16 = mybir.dt.bfloat16
    i16 = mybir.dt.int16

    qt_d = nc.dram_tensor("qt", [DUOS * 2 * D, S], bf16, kind="ExternalInput").ap()
    kt_d = nc.dram_tensor("kt", [DUOS * 2 * D, S], bf16, kind="ExternalInput").ap()
    v_d = nc.dram_tensor("v", [PAIRS * S, D], bf16, kind="ExternalInput").ap()
    out_d = nc.dram_tensor("out", [PAIRS * S, D], f32, kind="ExternalOutput").ap()

    with tile.TileContext(nc) as tc:
        with (
            tc.tile_pool(name="consts", bufs=1) as consts,
            tc.tile_pool(name="qk", bufs=2) as qk_pool,
            tc.tile_pool(name="vp", bufs=2) as v_pool,
            tc.tile_pool(name="pp", bufs=8) as p_pool,
            tc.tile_pool(name="op", bufs=3) as o_pool,
            tc.tile_pool(name="fp", bufs=3) as f_pool,
            tc.tile_pool(name="rp", bufs=3) as r_pool,
            tc.tile_pool(name="big", bufs=3, space="PSUM") as big_pool,
            tc.tile_pool(name="acc", bufs=2, space="PSUM") as acc_pool,
        ):
            ident = consts.tile([D + 1, D + 1], bf16)
            make_identity(nc, ident)
            st = {"exp": 0, "tail": 0}

            def load_duo(dd):
                qsb = qk_pool.tile([2 * D, S], bf16, tag="qsb", name=f"q{dd}")
                ksb = qk_pool.tile([2 * D, S], bf16, tag="ksb", name=f"k{dd}")
                r0, r1 = dd * 128, (dd + 1) * 128
                if dd == 0:
                    # first duo: land the j=0 columns first so the PE starts;
                    # the k chunk rides the idle vector queue at startup
                    nc.sync.dma_start(out=qsb[:, 0:QT], in_=qt_d[r0:r1, 0:QT])
                    nc.scalar.dma_start(out=ksb[:, 0:QT],
                                        in_=kt_d[r0:r1, 0:QT])
                    nc.sync.dma_start(out=qsb[:, QT:], in_=qt_d[r0:r1, QT:])
                    nc.sync.dma_start(out=ksb[:, QT:], in_=kt_d[r0:r1, QT:])
                else:
                    nc.sync.dma_start(out=qsb, in_=qt_d[r0:r1, :])
                    nc.sync.dma_start(out=ksb, in_=kt_d[r0:r1, :])
                vs = []
                for s_ in range(2):
                    h = 2 * dd + s_
                    vt = v_pool.tile([KT, S // KT, D + 1], bf16, tag=f"v{s_}",
                                     name=f"v{dd}_{s_}")
                    nc.gpsimd.memset(vt[:, :, D:D + 1], 1.0)
                    nc.sync.dma_start(
                        out=vt[:, :, 0:D],
                        in_=v_d[h * S:(h + 1) * S, :].rearrange(
                            "(t kp) d -> kp t d", kp=KT),
                    )
                    vs.append(vt)
                return qsb, ksb, vs

            def emit_s(sb, j, t):
                qsb, ksb, _ = sb
                off = max(0, KT * t - QT * j)
                w = QT - off
                q0 = QT * j + off
                ps = big_pool.tile([KT, 2, QT], f32, tag="ps", name="ps")
                nc.tensor.matmul(
                    ps[:, 0, 0:w],
                    lhsT=ksb[0:D, KT * t:KT * (t + 1)],
                    rhs=qsb[0:D, q0:QT * (j + 1)],
                    start=True, stop=True, tile_position=(0, 0),
                )
                nc.tensor.matmul(
                    ps[:, 1, 0:w],
                    lhsT=ksb[D:2 * D, KT * t:KT * (t + 1)],
                    rhs=qsb[D:2 * D, q0:QT * (j + 1)],
                    start=True, stop=True, tile_position=(64, 0),
                )
                p3 = p_pool.tile([KT, 2, QT], bf16, tag="p3", name="p3")
                eng = EXP_PATTERN[st["exp"] % len(EXP_PATTERN)]
                st["exp"] += 1
                if eng == "s":
                    nc.scalar.activation(
                        out=p3[:, :, 0:w], in_=ps[:, :, 0:w],
                        func=mybir.ActivationFunctionType.Exp, scale=SCALE,
                    )
                else:
                    nc.vector.tensor_scalar(
                        out=p3[:, :, 0:w].bitcast(i16), in0=ps[:, :, 0:w],
                        scalar1=A_SCH, scalar2=B_SCH,
                        op0=mybir.AluOpType.mult, op1=mybir.AluOpType.add,
                    )
                if t >= (QT // KT) * j:  # diagonal tile: zero q_rel < k_rel
                    nc.gpsimd.affine_select(
                        out=p3[:, :, 0:KT], in_=p3[:, :, 0:KT],
                        compare_op=mybir.AluOpType.is_ge,
                        fill=0.0, base=0,
                        pattern=[[0, 2], [1, KT]], channel_multiplier=-1,
                    )
                return p3, off, w

            def emit_pv(sb, accs, t, p3off, first, last):
                p3, off, w = p3off
                for s_ in range(2):
                    nc.tensor.matmul(
                        accs[s_][:, off:QT],
                        lhsT=sb[2][s_][:, t, :],
                        rhs=p3[:, s_, 0:w],
                        start=first, stop=last,
                    )

            def emit_tail(h, j, acc):
                osb = o_pool.tile([D + 1, QT], bf16, tag="osb", name="osb")
                if st["tail"] % 2 == 0:
                    nc.vector.tensor_copy(osb, acc)
                else:
                    nc.scalar.copy(out=osb, in_=acc)
                st["tail"] += 1
                # transpose scratch carved from a ps slot's first bank
                pst = big_pool.tile([KT, 2, QT], f32, tag="ps", name="tpslot")
                tp = pst[:, 0, 0:(QT // KT) * (D + 1)].rearrange(
                    "p (b c) -> p b c", b=QT // KT)
                for b_ in range(QT // KT):
                    nc.tensor.matmul(
                        tp[:, b_, :],
                        lhsT=osb[:, KT * b_:KT * (b_ + 1)],
                        rhs=ident, start=True, stop=True,
                    )
                rinv = r_pool.tile([KT, QT // KT], f32, tag="ri", name="rinv")
                nc.vector.reciprocal(rinv, tp[:, :, D])
                fsb = f_pool.tile([KT, QT // KT, D], f32, tag="f", name="fsb")
                # one mul for all 4 blocks: rinv broadcast along d (stride 0)
                nc.vector.tensor_tensor(
                    out=fsb, in0=tp[:, :, 0:D],
                    in1=rinv.broadcast_to([KT, QT // KT, D]),
                    op=mybir.AluOpType.mult,
                )
                row0 = h * S + QT * j
                nc.sync.dma_start(
                    out=out_d[row0:row0 + QT, :].rearrange(
                        "(b p) d -> p b d", p=KT),
                    in_=fsb,
                )

            sbs = load_duo(0)
            sbs_next = None
            LAG = GR + 1
            pend = []           # (sb, accs, nkt, t, p3off, uid)
            npop = {}           # uid -> PVs emitted so far
            pending_tails = []  # (uid, head, j, acc)

            def pop_pv():
                psb, paccs, pnkt, tt, p3off, uid = pend.pop(0)
                k = npop.get(uid, 0)
                emit_pv(psb, paccs, tt, p3off,
                        first=(k == 0), last=(k == pnkt - 1))
                npop[uid] = k + 1

            def emit_ready_tails():
                # a unit's tails may go once all its PVs are emitted; placed
                # at group boundaries so the transpose matmuls never land
                # mid-PV-run (which breaks MM chaining)
                while pending_tails and npop.get(pending_tails[0][0], 0) == \
                        4 * (pending_tails[0][2] + 1):
                    emit_tail(*pending_tails.pop(0)[1:])

            for dd in range(DUOS):
                sb = sbs
                for j in range(NQT):
                    uid = dd * NQT + j
                    nkt = (QT // KT) * (j + 1)
                    accx = acc_pool.tile([D + 1, QT], f32, tag="acc",
                                         name="accx")
                    accy = acc_pool.tile([D + 1, QT], f32, tag="acc",
                                         name="accy")
                    accs = (accx, accy)
                    # diagonal k-tiles first (longest exp->mask chain)
                    t_order = list(range(4 * j, nkt)) + list(range(4 * j))
                    for g in range(nkt // GR):
                        for half in range(GR):
                            t = t_order[GR * g + half]
                            pend.append((sb, accs, nkt, t,
                                         emit_s(sb, j, t), uid))
                        if g == 0 and j == 1 and dd + 1 < DUOS:
                            sbs_next = load_duo(dd + 1)
                        while len(pend) > LAG:
                            pop_pv()
                        emit_ready_tails()
                    pending_tails.append((uid, 2 * dd, j, accx))
                    pending_tails.append((uid, 2 * dd + 1, j, accy))
                sbs = sbs_next
            while pend:
                pop_pv()
            for args in pending_tails:
                emit_tail(*args[1:])

    nc.compile()
    return nc


def _get_nc():
    if "nc" not in _COMPILED:
        _COMPILED["nc"] = build_nc()
    return _COMPILED["nc"]


def make_in_maps(q, k, v):
    q = np.asarray(q, dtype=np.float32).reshape(B * H, S, D)
    k = np.asarray(k, dtype=np.float32).reshape(B * H, S, D)
    v = np.asarray(v, dtype=np.float32).reshape(B * H, S, D)
    in_maps = []
    for c in range(N_CORES):
        sl = slice(c * PAIRS, (c + 1) * PAIRS)
        # duo-stacked d-major [DUOS*128, S]: duo dd rows 0-63 = head 2dd,
        # rows 64-127 = head 2dd+1
        qt = np.ascontiguousarray(q[sl].transpose(0, 2, 1)).reshape(
            DUOS * 2 * D, S)
        kt = np.ascontiguousarray(k[sl].transpose(0, 2, 1)).reshape(
            DUOS * 2 * D, S)
        in_maps.append({
            "qt": qt.astype(BF16),
            "kt": kt.astype(BF16),
            "v": np.ascontiguousarray(v[sl]).reshape(PAIRS * S, D).astype(BF16),
        })
    return in_maps


def assemble(results):
    out = np.empty((B * H, S, D), dtype=np.float32)
    for c in range(N_CORES):
        out[c * PAIRS:(c + 1) * PAIRS] = results[c]["out"].reshape(PAIRS, S, D)
    return np.ascontiguousarray(
        out.reshape(B, H, S, D).transpose(0, 2, 1, 3).reshape(B, S, H * D))


def kernel(q, k, v):
    nc = _get_nc()
    res = bass_utils.run_bass_kernel_spmd(
        nc, make_in_maps(q, k, v), core_ids=list(range(N_CORES)))
    return assemble(res.results)


# revision 35
# speedup vs baseline: 1.0838x; 1.0335x over previous
"""Causal multi-head attention (B=4, H=16, S=2048, D=64) on 8 TRN2 NeuronCores.

Sharding: 64 (batch, head) pairs, 8 per core, processed as 4 "duos" (X, Y).
q/k are host-pre-transposed to d-major and duo-stacked: X's 64 d-rows on
SBUF partitions 0-63, Y's on 64-127.

Per-duo algorithm (flash-attention, transposed-score layout), per q-tile
("unit", 512 q cols), over k-tiles t in the causal lower triangle:

S^T stage - ROW-TILED matmul pairs: X's S^T at tile_position (0,0) using PE
rows 0-63, Y's at (64,0) using rows 64-127. The two contraction-64 matmuls
run concurrently in the PE array, writing one [128, 2, 512] two-bank PSUM
super-tile; full-array activity also keeps the HAM clock monitor at K=8/8
(2.4 GHz) - half-array streams measurably never leave the 1.2 GHz cold
clock (the v1 kernel's core defect: every matmul ran at (219+N)/1.2 ns).

exp - one wide instruction per k-tile covering both streams ((N+352)/1.2 ns
on ScalarE: batching amortizes the 352-cycle overhead). ScalarE alone
(~46us/duo) would cap the kernel below the PE's ~30us/duo, so k-tiles
alternate between ScalarE exact exp and a VectorE Schraudolph bit-trick
exp: int16(round(x*128*log2e*scale + (16256 - 128*0.043677))) bitcast to
bf16 ~= exp(x*scale) within +-3 percent (measured end-to-end rel-err 0.010
at a 0.02 gate; all-ScalarE would be 0.003). EXP_PATTERN must alternate
strictly - two adjacent "s" slots overload ScalarE within one pipeline
group and cost ~4-30us. Causal masks: a single GPSIMD affine_select over
both streams' first 128 columns of each diagonal tile; diagonal k-tiles
are emitted FIRST in each unit so their longer exp->mask chain never lands
on the end-of-unit PV flush (PV summation order is commutative).

PV stage - acc[65, 512] += V'[128, 65]^T @ P[128, w] per stream; V' carries
a ones column so acc row 64 accumulates the softmax denominator for free.
PV-pair emission lags the S-pairs by one GR=4 group so the in-order PE
queue always holds work that is independent of pending exps (3 ps
super-tile buffers cover the lag; the tail transpose scratch is carved from
a ps slot's first bank to stay within 8 PSUM banks total).

Unit tails (deferred into the next unit's first groups, X/Y staggered):
evict acc -> SBUF bf16 (alternating ScalarE/VectorE), transpose via 4
identity matmuls into one PSUM bank [128, 4, 65], one strided reciprocal of
the 4 denominator columns, one broadcast tensor_tensor normalize mul, one
DMA of [512, 64] fp32 to DRAM in natural [q, d] layout.

The first duo's q/k DMAs land the j=0 columns first so the PE starts ~2us
earlier; per-engine-queue DMA spreading measurably backfires (a dma_start
occupies its issuing engine for the whole transfer).
"""

import math

import numpy as np
import ml_dtypes

import concourse.bass as bass
import concourse.bacc as bacc
import concourse.tile as tile
import concourse.mybir as mybir
from concourse import bass_utils
from concourse.masks import make_identity

B, H, S, D = 4, 16, 2048, 64
N_CORES = 8
PAIRS = (B * H) // N_CORES  # 8 heads per core
DUOS = PAIRS // 2           # 4 lockstep duos per core
QT = 512                    # q-tile width
KT = 128                    # k-tile rows
NQT = S // QT               # 4 q-tiles per head
GR = 4                      # k-tiles per pipeline group
SCALE = 1.0 / math.sqrt(D)
A_SCH = (128.0 / math.log(2.0)) * SCALE       # Schraudolph slope (scale folded)
B_SCH = 16256.0 - 128.0 * 0.043677            # Schraudolph offset (bf16 bias)
EXP_PATTERN = ("s", "v", "s", "v", "s", "v", "s", "v", "s")  # 5/9 ScalarE exact
BF16 = ml_dtypes.bfloat16

_COMPILED = {}


def build_nc():
    nc = bacc.Bacc(
        "TRN2",
        target_bir_lowering=False,
        debug=False,
        enable_asserts=True,
        num_devices=N_CORES,
    )
    f32 = mybir.dt.float32
    bf16 = mybir.dt.bfloat16
    i16 = mybir.dt.int16

    qt_d = nc.dram_tensor("qt", [DUOS * 2 * D, S], bf16, kind="ExternalInput").ap()
    kt_d = nc.dram_tensor("kt", [DUOS * 2 * D, S], bf16, kind="ExternalInput").ap()
    v_d = nc.dram_tensor("v", [PAIRS * S, D], bf16, kind="ExternalInput").ap()
    out_d = nc.dram_tensor("out", [PAIRS * S, D], f32, kind="ExternalOutput").ap()

    with tile.TileContext(nc) as tc:
        with (
            tc.tile_pool(name="consts", bufs=1) as consts,
            tc.tile_pool(name="qk", bufs=2) as qk_pool,
            tc.tile_pool(name="vp", bufs=2) as v_pool,
            tc.tile_pool(name="pp", bufs=8) as p_pool,
            tc.tile_pool(name="op", bufs=3) as o_pool,
            tc.tile_pool(name="fp", bufs=3) as f_pool,
            tc.tile_pool(name="rp", bufs=3) as r_pool,
            tc.tile_pool(name="big", bufs=3, space="PSUM") as big_pool,
            tc.tile_pool(name="acc", bufs=2, space="PSUM") as acc_pool,
        ):
            ident = consts.tile([D + 1, D + 1], bf16)
            make_identity(nc, ident)
            st = {"exp": 0, "tail": 0}

            def load_duo(dd):
                qsb = qk_pool.tile([2 * D, S], bf16, tag="qsb", name=f"q{dd}")
                ksb = qk_pool.tile([2 * D, S], bf16, tag="ksb", name=f"k{dd}")
                r0, r1 = dd * 128, (dd + 1) * 128
                if dd == 0:
                    # first duo: land the j=0 columns first so the PE starts;
                    # the k chunk rides the idle vector queue at startup
                    nc.sync.dma_start(out=qsb[:, 0:QT], in_=qt_d[r0:r1, 0:QT])
                    nc.scalar.dma_start(out=ksb[:, 0:QT],
                                        in_=kt_d[r0:r1, 0:QT])
                    nc.sync.dma_start(out=qsb[:, QT:], in_=qt_d[r0:r1, QT:])
                    nc.sync.dma_start(out=ksb[:, QT:], in_=kt_d[r0:r1, QT:])
                else:
                    nc.sync.dma_start(out=qsb, in_=qt_d[r0:r1, :])
                    nc.sync.dma_start(out=ksb, in_=kt_d[r0:r1, :])
                vs = []
                for s_ in range(2):
                    h = 2 * dd + s_
                    vt = v_pool.tile([KT, S // KT, D + 1], bf16, tag=f"v{s_}",
                                     name=f"v{dd}_{s_}")
                    nc.gpsimd.memset(vt[:, :, D:D + 1], 1.0)
                    nc.sync.dma_start(
                        out=vt[:, :, 0:D],
                        in_=v_d[h * S:(h + 1) * S, :].rearrange(
                            "(t kp) d -> kp t d", kp=KT),
                    )
                    vs.append(vt)
                return qsb, ksb, vs

            def emit_s(sb, j, t):
                qsb, ksb, _ = sb
                off = max(0, KT * t - QT * j)
                w = QT - off
                q0 = QT * j + off
                ps = big_pool.tile([KT, 2, QT], f32, tag="ps", name="ps")
                nc.tensor.matmul(
                    ps[:, 0, 0:w],
                    lhsT=ksb[0:D, KT * t:KT * (t + 1)],
                    rhs=qsb[0:D, q0:QT * (j + 1)],
                    start=True, stop=True, tile_position=(0, 0),
                )
                nc.tensor.matmul(
                    ps[:, 1, 0:w],
                    lhsT=ksb[D:2 * D, KT * t:KT * (t + 1)],
                    rhs=qsb[D:2 * D, q0:QT * (j + 1)],
                    start=True, stop=True, tile_position=(64, 0),
                )
                p3 = p_pool.tile([KT, 2, QT], bf16, tag="p3", name="p3")
                eng = EXP_PATTERN[st["exp"] % len(EXP_PATTERN)]
                st["exp"] += 1
                if eng == "s":
                    nc.scalar.activation(
                        out=p3[:, :, 0:w], in_=ps[:, :, 0:w],
                        func=mybir.ActivationFunctionType.Exp, scale=SCALE,
                    )
                else:
                    nc.vector.tensor_scalar(
                        out=p3[:, :, 0:w].bitcast(i16), in0=ps[:, :, 0:w],
                        scalar1=A_SCH, scalar2=B_SCH,
                        op0=mybir.AluOpType.mult, op1=mybir.AluOpType.add,
                    )
                if t >= (QT // KT) * j:  # diagonal tile: zero q_rel < k_rel
                    nc.gpsimd.affine_select(
                        out=p3[:, :, 0:KT], in_=p3[:, :, 0:KT],
                        compare_op=mybir.AluOpType.is_ge,
                        fill=0.0, base=0,
                        pattern=[[0, 2], [1, KT]], channel_multiplier=-1,
                    )
                return p3, off, w

            def emit_pv(sb, accs, t, p3off, first, last):
                p3, off, w = p3off
                for s_ in range(2):
                    nc.tensor.matmul(
                        accs[s_][:, off:QT],
                        lhsT=sb[2][s_][:, t, :],
                        rhs=p3[:, s_, 0:w],
                        start=first, stop=last,
                    )

            def emit_tail(h, j, acc):
                osb = o_pool.tile([D + 1, QT], bf16, tag="osb", name="osb")
                if st["tail"] % 2 == 0:
                    nc.vector.tensor_copy(osb, acc)
                else:
                    nc.scalar.copy(out=osb, in_=acc)
                st["tail"] += 1
                # transpose scratch carved from a ps slot's first bank
                pst = big_pool.tile([KT, 2, QT], f32, tag="ps", name="tpslot")
                tp = pst[:, 0, 0:(QT // KT) * (D + 1)].rearrange(
                    "p (b c) -> p b c", b=QT // KT)
                for b_ in range(QT // KT):
                    nc.tensor.matmul(
                        tp[:, b_, :],
                        lhsT=osb[:, KT * b_:KT * (b_ + 1)],
                        rhs=ident, start=True, stop=True,
                    )
                rinv = r_pool.tile([KT, QT // KT], f32, tag="ri", name="rinv")
                nc.vector.reciprocal(rinv, tp[:, :, D])
                fsb = f_pool.tile([KT, QT // KT, D], f32, tag="f", name="fsb")
                # one mul for all 4 blocks: rinv broadcast along d (stride 0)
                nc.vector.tensor_tensor(
                    out=fsb, in0=tp[:, :, 0:D],
                    in1=rinv.broadcast_to([KT, QT // KT, D]),
                    op=mybir.AluOpType.mult,
                )
                row0 = h * S + QT * j
                nc.sync.dma_start(
                    out=out_d[row0:row0 + QT, :].rearrange(
                        "(b p) d -> p b d", p=KT),
                    in_=fsb,
                )

            sbs = load_duo(0)
            sbs_next = None
            LAG = GR + 1
            pend = []           # (sb, accs, nkt, t, p3off, uid)
            npop = {}           # uid -> PVs emitted so far
            pending_tails = []  # (uid, head, j, acc)

            def pop_pv():
                psb, paccs, pnkt, tt, p3off, uid = pend.pop(0)
                k = npop.get(uid, 0)
                emit_pv(psb, paccs, tt, p3off,
                        first=(k == 0), last=(k == pnkt - 1))
                npop[uid] = k + 1

            def emit_ready_tails():
                # a unit's tails may go once all its PVs are emitted; placed
                # at group boundaries so the transpose matmuls never land
                # mid-PV-run (which breaks MM chaining)
                while pending_tails and npop.get(pending_tails[0][0], 0) == \
                        4 * (pending_tails[0][2] + 1):
                    emit_tail(*pending_tails.pop(0)[1:])

            for dd in range(DUOS):
                sb = sbs
                for j in range(NQT):
                    uid = dd * NQT + j
                    nkt = (QT // KT) * (j + 1)
                    accx = acc_pool.tile([D + 1, QT], f32, tag="acc",
                                         name="accx")
                    accy = acc_pool.tile([D + 1, QT], f32, tag="acc",
                                         name="accy")
                    accs = (accx, accy)
                    # spread diagonal k-tiles evenly (one per group):
                    # front-loading all 4 serializes their exp->mask chains
                    # past what the PE can cover, and they must stay off the
                    # end-of-unit flush
                    fi = iter(range(4 * j))
                    t_order = []
                    for dt in range(4 * j, nkt):
                        t_order.append(dt)
                        for _ in range(j):
                            t_order.append(next(fi))
                    for g in range(nkt // GR):
                        for half in range(GR):
                            t = t_order[GR * g + half]
                            pend.append((sb, accs, nkt, t,
                                         emit_s(sb, j, t), uid))
                        if g == 0 and j == 1 and dd + 1 < DUOS:
                            sbs_next = load_duo(dd + 1)
                        while len(pend) > LAG:
                            pop_pv()
                        emit_ready_tails()
                    pending_tails.append((uid, 2 * dd, j, accx))
                    pending_tails.append((uid, 2 * dd + 1, j, accy))
                sbs = sbs_next
            while pend:
                pop_pv()
            for args in pending_tails:
                emit_tail(*args[1:])

    nc.compile()
    return nc


def _get_nc():
    if "nc" not in _COMPILED:
        _COMPILED["nc"] = build_nc()
    return _COMPILED["nc"]


def make_in_maps(q, k, v):
    q = np.asarray(q, dtype=np.float32).reshape(B * H, S, D)
    k = np.asarray(k, dtype=np.float32).reshape(B * H, S, D)
    v = np.asarray(v, dtype=np.float32).reshape(B * H, S, D)
    in_maps = []
    for c in range(N_CORES):
        sl = slice(c * PAIRS, (c + 1) * PAIRS)
        # duo-stacked d-major [DUOS*128, S]: duo dd rows 0-63 = head 2dd,
        # rows 64-127 = head 2dd+1
        qt = np.ascontiguousarray(q[sl].transpose(0, 2, 1)).reshape(
            DUOS * 2 * D, S)
        kt = np.ascontiguousarray(k[sl].transpose(0, 2, 1)).reshape(
            DUOS * 2 * D, S)
        in_maps.append({
            "qt": qt.astype(BF16),
            "kt": kt.astype(BF16),
            "v": np.ascontiguousarray(v[sl]).reshape(PAIRS * S, D).astype(BF16),
        })
    return in_maps


def assemble(results):
    out = np.empty((B * H, S, D), dtype=np.float32)
    for c in range(N_CORES):
        out[c * PAIRS:(c + 1) * PAIRS] = results[c]["out"].reshape(PAIRS, S, D)
    return np.ascontiguousarray(
        out.reshape(B, H, S, D).transpose(0, 2, 1, 3).reshape(B, S, H * D))


def kernel(q, k, v):
    nc = _get_nc()
    res = bass_utils.run_bass_kernel_spmd(
        nc, make_in_maps(q, k, v), core_ids=list(range(N_CORES)))
    return assemble(res.results)


# revision 36
# speedup vs baseline: 1.2707x; 1.1724x over previous
"""Causal multi-head attention (B=4, H=16, S=2048, D=64) on 8 TRN2 NeuronCores.

Sharding: 64 (batch, head) pairs, 8 per core, processed as 4 "duos" (X, Y).
q/k are host-pre-transposed to d-major and duo-stacked: X's 64 d-rows on
SBUF partitions 0-63, Y's on 64-127.

Per-duo algorithm (flash-attention, transposed-score layout), per q-tile
("unit", 512 q cols), over k-tiles t in the causal lower triangle:

S^T stage - ROW-TILED matmul pairs: X's S^T at tile_position (0,0) using PE
rows 0-63, Y's at (64,0) using rows 64-127. The two contraction-64 matmuls
run concurrently in the PE array, writing one [128, 2, 512] two-bank PSUM
super-tile; full-array activity also keeps the HAM clock monitor at K=8/8
(2.4 GHz) - half-array streams measurably never leave the 1.2 GHz cold
clock (the v1 kernel's core defect: every matmul ran at (219+N)/1.2 ns).

exp - one wide instruction per k-tile covering both streams ((N+352)/1.2 ns
on ScalarE: batching amortizes the 352-cycle overhead). ScalarE alone
(~46us/duo) would cap the kernel below the PE's ~30us/duo, so k-tiles
alternate between ScalarE exact exp and a VectorE Schraudolph bit-trick
exp: int16(round(x*128*log2e*scale + (16256 - 128*0.043677))) bitcast to
bf16 ~= exp(x*scale) within +-3 percent (measured end-to-end rel-err 0.010
at a 0.02 gate; all-ScalarE would be 0.003). EXP_PATTERN must alternate
strictly - two adjacent "s" slots overload ScalarE within one pipeline
group and cost ~4-30us. Causal masks: a single GPSIMD affine_select over
both streams' first 128 columns of each diagonal tile; diagonal k-tiles
are emitted FIRST in each unit so their longer exp->mask chain never lands
on the end-of-unit PV flush (PV summation order is commutative).

PV stage - acc[65, 512] += V'[128, 65]^T @ P[128, w] per stream; V' carries
a ones column so acc row 64 accumulates the softmax denominator for free.
PV-pair emission lags the S-pairs by one GR=4 group so the in-order PE
queue always holds work that is independent of pending exps (3 ps
super-tile buffers cover the lag; the tail transpose scratch is carved from
a ps slot's first bank to stay within 8 PSUM banks total).

Unit tails (deferred into the next unit's first groups, X/Y staggered):
evict acc -> SBUF bf16 (alternating ScalarE/VectorE), transpose via 4
identity matmuls into one PSUM bank [128, 4, 65], one strided reciprocal of
the 4 denominator columns, one broadcast tensor_tensor normalize mul, one
DMA of [512, 64] fp32 to DRAM in natural [q, d] layout.

The first duo's q/k DMAs land the j=0 columns first so the PE starts ~2us
earlier; per-engine-queue DMA spreading measurably backfires (a dma_start
occupies its issuing engine for the whole transfer).
"""

import math

import numpy as np
import ml_dtypes

import concourse.bass as bass
import concourse.bacc as bacc
import concourse.tile as tile
import concourse.mybir as mybir
from concourse import bass_utils
from concourse.masks import make_identity

B, H, S, D = 4, 16, 2048, 64
N_CORES = 8
PAIRS = (B * H) // N_CORES  # 8 heads per core
DUOS = PAIRS // 2           # 4 lockstep duos per core
QT = 512                    # q-tile width
KT = 128                    # k-tile rows
NQT = S // QT               # 4 q-tiles per head
GR = 4                      # k-tiles per pipeline group
SCALE = 1.0 / math.sqrt(D)
A_SCH = (128.0 / math.log(2.0)) * SCALE       # Schraudolph slope (scale folded)
B_SCH = 16256.0 - 128.0 * 0.043677            # Schraudolph offset (bf16 bias)
EXP_PATTERN = ("s", "v", "s", "v", "s", "v", "s", "v", "s")  # 5/9 ScalarE exact
BF16 = ml_dtypes.bfloat16

_COMPILED = {}


def build_nc():
    nc = bacc.Bacc(
        "TRN2",
        target_bir_lowering=False,
        debug=False,
        enable_asserts=True,
        num_devices=N_CORES,
    )
    f32 = mybir.dt.float32
    bf16 = mybir.dt.bfloat16
    i16 = mybir.dt.int16

    qt_d = nc.dram_tensor("qt", [DUOS * 2 * D, S], bf16, kind="ExternalInput").ap()
    kt_d = nc.dram_tensor("kt", [DUOS * 2 * D, S], bf16, kind="ExternalInput").ap()
    v_d = nc.dram_tensor("v", [PAIRS * S, D], bf16, kind="ExternalInput").ap()
    out_d = nc.dram_tensor("out", [PAIRS * S, D], f32, kind="ExternalOutput").ap()

    with tile.TileContext(nc) as tc:
        with (
            tc.tile_pool(name="consts", bufs=1) as consts,
            tc.tile_pool(name="qk", bufs=2) as qk_pool,
            tc.tile_pool(name="vp", bufs=2) as v_pool,
            tc.tile_pool(name="pp", bufs=8) as p_pool,
            tc.tile_pool(name="op", bufs=3) as o_pool,
            tc.tile_pool(name="fp", bufs=3) as f_pool,
            tc.tile_pool(name="rp", bufs=3) as r_pool,
            tc.tile_pool(name="big", bufs=3, space="PSUM") as big_pool,
            tc.tile_pool(name="acc", bufs=2, space="PSUM") as acc_pool,
        ):
            ident = consts.tile([D + 1, D + 1], bf16)
            make_identity(nc, ident)
            # 0/1 causal triangle for DVE-side diagonal masking
            trimask = consts.tile([KT, 2, KT], bf16)
            nc.gpsimd.memset(trimask, 1.0)
            nc.gpsimd.affine_select(
                out=trimask, in_=trimask,
                compare_op=mybir.AluOpType.is_ge, fill=0.0, base=0,
                pattern=[[0, 2], [1, KT]], channel_multiplier=-1,
            )
            st = {"exp": 0, "tail": 0}

            def load_duo(dd):
                qsb = qk_pool.tile([2 * D, S], bf16, tag="qsb", name=f"q{dd}")
                ksb = qk_pool.tile([2 * D, S], bf16, tag="ksb", name=f"k{dd}")
                r0, r1 = dd * 128, (dd + 1) * 128
                if dd == 0:
                    # first duo: land the j=0 columns first so the PE starts;
                    # the k chunk rides the idle vector queue at startup
                    nc.sync.dma_start(out=qsb[:, 0:QT], in_=qt_d[r0:r1, 0:QT])
                    nc.scalar.dma_start(out=ksb[:, 0:QT],
                                        in_=kt_d[r0:r1, 0:QT])
                    nc.sync.dma_start(out=qsb[:, QT:], in_=qt_d[r0:r1, QT:])
                    nc.sync.dma_start(out=ksb[:, QT:], in_=kt_d[r0:r1, QT:])
                else:
                    nc.sync.dma_start(out=qsb, in_=qt_d[r0:r1, :])
                    nc.sync.dma_start(out=ksb, in_=kt_d[r0:r1, :])
                vs = []
                for s_ in range(2):
                    h = 2 * dd + s_
                    vt = v_pool.tile([KT, S // KT, D + 1], bf16, tag=f"v{s_}",
                                     name=f"v{dd}_{s_}")
                    nc.gpsimd.memset(vt[:, :, D:D + 1], 1.0)
                    nc.sync.dma_start(
                        out=vt[:, :, 0:D],
                        in_=v_d[h * S:(h + 1) * S, :].rearrange(
                            "(t kp) d -> kp t d", kp=KT),
                    )
                    vs.append(vt)
                return qsb, ksb, vs

            def emit_s(sb, j, t):
                qsb, ksb, _ = sb
                off = max(0, KT * t - QT * j)
                w = QT - off
                q0 = QT * j + off
                ps = big_pool.tile([KT, 2, QT], f32, tag="ps", name="ps")
                nc.tensor.matmul(
                    ps[:, 0, 0:w],
                    lhsT=ksb[0:D, KT * t:KT * (t + 1)],
                    rhs=qsb[0:D, q0:QT * (j + 1)],
                    start=True, stop=True, tile_position=(0, 0),
                )
                nc.tensor.matmul(
                    ps[:, 1, 0:w],
                    lhsT=ksb[D:2 * D, KT * t:KT * (t + 1)],
                    rhs=qsb[D:2 * D, q0:QT * (j + 1)],
                    start=True, stop=True, tile_position=(64, 0),
                )
                p3 = p_pool.tile([KT, 2, QT], bf16, tag="p3", name="p3")
                eng = EXP_PATTERN[st["exp"] % len(EXP_PATTERN)]
                st["exp"] += 1
                if eng == "s":
                    nc.scalar.activation(
                        out=p3[:, :, 0:w], in_=ps[:, :, 0:w],
                        func=mybir.ActivationFunctionType.Exp, scale=SCALE,
                    )
                else:
                    nc.vector.tensor_scalar(
                        out=p3[:, :, 0:w].bitcast(i16), in0=ps[:, :, 0:w],
                        scalar1=A_SCH, scalar2=B_SCH,
                        op0=mybir.AluOpType.mult, op1=mybir.AluOpType.add,
                    )
                if t >= (QT // KT) * j:  # diagonal tile: zero q_rel < k_rel
                    if eng == "v":
                        # same engine as the exp: no cross-engine hop, and
                        # it halves the gpsimd mask serialization
                        nc.vector.tensor_tensor(
                            out=p3[:, :, 0:KT], in0=p3[:, :, 0:KT],
                            in1=trimask, op=mybir.AluOpType.mult,
                        )
                    else:
                        nc.gpsimd.affine_select(
                            out=p3[:, :, 0:KT], in_=p3[:, :, 0:KT],
                            compare_op=mybir.AluOpType.is_ge,
                            fill=0.0, base=0,
                            pattern=[[0, 2], [1, KT]], channel_multiplier=-1,
                        )
                return p3, off, w

            def emit_pv(sb, accs, t, p3off, first, last):
                p3, off, w = p3off
                for s_ in range(2):
                    nc.tensor.matmul(
                        accs[s_][:, off:QT],
                        lhsT=sb[2][s_][:, t, :],
                        rhs=p3[:, s_, 0:w],
                        start=first, stop=last,
                    )

            def emit_tail(h, j, acc):
                osb = o_pool.tile([D + 1, QT], bf16, tag="osb", name="osb")
                if st["tail"] % 2 == 0:
                    nc.vector.tensor_copy(osb, acc)
                else:
                    nc.scalar.copy(out=osb, in_=acc)
                st["tail"] += 1
                # transpose scratch carved from a ps slot's first bank
                pst = big_pool.tile([KT, 2, QT], f32, tag="ps", name="tpslot")
                tp = pst[:, 0, 0:(QT // KT) * (D + 1)].rearrange(
                    "p (b c) -> p b c", b=QT // KT)
                for b_ in range(QT // KT):
                    nc.tensor.matmul(
                        tp[:, b_, :],
                        lhsT=osb[:, KT * b_:KT * (b_ + 1)],
                        rhs=ident, start=True, stop=True,
                    )
                rinv = r_pool.tile([KT, QT // KT], f32, tag="ri", name="rinv")
                nc.vector.reciprocal(rinv, tp[:, :, D])
                fsb = f_pool.tile([KT, QT // KT, D], f32, tag="f", name="fsb")
                # one mul for all 4 blocks: rinv broadcast along d (stride 0)
                nc.vector.tensor_tensor(
                    out=fsb, in0=tp[:, :, 0:D],
                    in1=rinv.broadcast_to([KT, QT // KT, D]),
                    op=mybir.AluOpType.mult,
                )
                row0 = h * S + QT * j
                nc.sync.dma_start(
                    out=out_d[row0:row0 + QT, :].rearrange(
                        "(b p) d -> p b d", p=KT),
                    in_=fsb,
                )

            sbs = load_duo(0)
            sbs_next = None
            LAG = GR + 1
            pend = []           # (sb, accs, nkt, t, p3off, uid)
            npop = {}           # uid -> PVs emitted so far
            pending_tails = []  # (uid, head, j, acc)

            def pop_pv():
                psb, paccs, pnkt, tt, p3off, uid = pend.pop(0)
                k = npop.get(uid, 0)
                emit_pv(psb, paccs, tt, p3off,
                        first=(k == 0), last=(k == pnkt - 1))
                npop[uid] = k + 1

            def emit_ready_tails():
                # a unit's tails may go once all its PVs are emitted; placed
                # at group boundaries so the transpose matmuls never land
                # mid-PV-run (which breaks MM chaining)
                while pending_tails and npop.get(pending_tails[0][0], 0) == \
                        4 * (pending_tails[0][2] + 1):
                    emit_tail(*pending_tails.pop(0)[1:])

            for dd in range(DUOS):
                sb = sbs
                for j in range(NQT):
                    uid = dd * NQT + j
                    nkt = (QT // KT) * (j + 1)
                    accx = acc_pool.tile([D + 1, QT], f32, tag="acc",
                                         name="accx")
                    accy = acc_pool.tile([D + 1, QT], f32, tag="acc",
                                         name="accy")
                    accs = (accx, accy)
                    # spread diagonal k-tiles evenly (one per group):
                    # front-loading all 4 serializes their exp->mask chains
                    # past what the PE can cover, and they must stay off the
                    # end-of-unit flush
                    fi = iter(range(4 * j))
                    t_order = []
                    for dt in range(4 * j, nkt):
                        t_order.append(dt)
                        for _ in range(j):
                            t_order.append(next(fi))
                    for g in range(nkt // GR):
                        for half in range(GR):
                            t = t_order[GR * g + half]
                            pend.append((sb, accs, nkt, t,
                                         emit_s(sb, j, t), uid))
                        if g == 0 and j == 1 and dd + 1 < DUOS:
                            sbs_next = load_duo(dd + 1)
                        while len(pend) > LAG:
                            pop_pv()
                        emit_ready_tails()
                    pending_tails.append((uid, 2 * dd, j, accx))
                    pending_tails.append((uid, 2 * dd + 1, j, accy))
                sbs = sbs_next
            while pend:
                pop_pv()
            for args in pending_tails:
                emit_tail(*args[1:])

    nc.compile()
    return nc


def _get_nc():
    if "nc" not in _COMPILED:
        _COMPILED["nc"] = build_nc()
    return _COMPILED["nc"]


def make_in_maps(q, k, v):
    q = np.asarray(q, dtype=np.float32).reshape(B * H, S, D)
    k = np.asarray(k, dtype=np.float32).reshape(B * H, S, D)
    v = np.asarray(v, dtype=np.float32).reshape(B * H, S, D)
    in_maps = []
    for c in range(N_CORES):
        sl = slice(c * PAIRS, (c + 1) * PAIRS)
        # duo-stacked d-major [DUOS*128, S]: duo dd rows 0-63 = head 2dd,
        # rows 64-127 = head 2dd+1
        qt = np.ascontiguousarray(q[sl].transpose(0, 2, 1)).reshape(
            DUOS * 2 * D, S)
        kt = np.ascontiguousarray(k[sl].transpose(0, 2, 1)).reshape(
            DUOS * 2 * D, S)
        in_maps.append({
            "qt": qt.astype(BF16),
            "kt": kt.astype(BF16),
            "v": np.ascontiguousarray(v[sl]).reshape(PAIRS * S, D).astype(BF16),
        })
    return in_maps


def assemble(results):
    out = np.empty((B * H, S, D), dtype=np.float32)
    for c in range(N_CORES):
        out[c * PAIRS:(c + 1) * PAIRS] = results[c]["out"].reshape(PAIRS, S, D)
    return np.ascontiguousarray(
        out.reshape(B, H, S, D).transpose(0, 2, 1, 3).reshape(B, S, H * D))


def kernel(q, k, v):
    nc = _get_nc()
    res = bass_utils.run_bass_kernel_spmd(
        nc, make_in_maps(q, k, v), core_ids=list(range(N_CORES)))
    return assemble(res.results)


# revision 37
# speedup vs baseline: 1.2836x; 1.0101x over previous
"""Causal multi-head attention (B=4, H=16, S=2048, D=64) on 8 TRN2 NeuronCores.

Sharding: 64 (batch, head) pairs, 8 per core, processed as 4 "duos" (X, Y).
q/k are host-pre-transposed to d-major and duo-stacked: X's 64 d-rows on
SBUF partitions 0-63, Y's on 64-127.

Per-duo algorithm (flash-attention, transposed-score layout), per q-tile
("unit", 512 q cols), over k-tiles t in the causal lower triangle:

S^T stage - ROW-TILED matmul pairs: X's S^T at tile_position (0,0) using PE
rows 0-63, Y's at (64,0) using rows 64-127. The two contraction-64 matmuls
run concurrently in the PE array, writing one [128, 2, 512] two-bank PSUM
super-tile; full-array activity also keeps the HAM clock monitor at K=8/8
(2.4 GHz) - half-array streams measurably never leave the 1.2 GHz cold
clock (the v1 kernel's core defect: every matmul ran at (219+N)/1.2 ns).

exp - one wide instruction per k-tile covering both streams ((N+352)/1.2 ns
on ScalarE: batching amortizes the 352-cycle overhead). ScalarE alone
(~46us/duo) would cap the kernel below the PE's ~30us/duo, so k-tiles
alternate between ScalarE exact exp and a VectorE Schraudolph bit-trick
exp: int16(round(x*128*log2e*scale + (16256 - 128*0.043677))) bitcast to
bf16 ~= exp(x*scale) within +-3 percent (measured end-to-end rel-err 0.010
at a 0.02 gate; all-ScalarE would be 0.003). EXP_PATTERN must alternate
strictly - two adjacent "s" slots overload ScalarE within one pipeline
group and cost ~4-30us. Causal masks: a single GPSIMD affine_select over
both streams' first 128 columns of each diagonal tile; diagonal k-tiles
are emitted FIRST in each unit so their longer exp->mask chain never lands
on the end-of-unit PV flush (PV summation order is commutative).

PV stage - acc[65, 512] += V'[128, 65]^T @ P[128, w] per stream; V' carries
a ones column so acc row 64 accumulates the softmax denominator for free.
PV-pair emission lags the S-pairs by one GR=4 group so the in-order PE
queue always holds work that is independent of pending exps (3 ps
super-tile buffers cover the lag; the tail transpose scratch is carved from
a ps slot's first bank to stay within 8 PSUM banks total).

Unit tails (deferred into the next unit's first groups, X/Y staggered):
evict acc -> SBUF bf16 (alternating ScalarE/VectorE), transpose via 4
identity matmuls into one PSUM bank [128, 4, 65], one strided reciprocal of
the 4 denominator columns, one broadcast tensor_tensor normalize mul, one
DMA of [512, 64] fp32 to DRAM in natural [q, d] layout.

The first duo's q/k DMAs land the j=0 columns first so the PE starts ~2us
earlier; per-engine-queue DMA spreading measurably backfires (a dma_start
occupies its issuing engine for the whole transfer).
"""

import math

import numpy as np
import ml_dtypes

import concourse.bass as bass
import concourse.bacc as bacc
import concourse.tile as tile
import concourse.mybir as mybir
from concourse import bass_utils
from concourse.masks import make_identity

B, H, S, D = 4, 16, 2048, 64
N_CORES = 8
PAIRS = (B * H) // N_CORES  # 8 heads per core
DUOS = PAIRS // 2           # 4 lockstep duos per core
QT = 512                    # q-tile width
KT = 128                    # k-tile rows
NQT = S // QT               # 4 q-tiles per head
GR = 4                      # k-tiles per pipeline group
SCALE = 1.0 / math.sqrt(D)
A_SCH = (128.0 / math.log(2.0)) * SCALE       # Schraudolph slope (scale folded)
B_SCH = 16256.0 - 128.0 * 0.043677            # Schraudolph offset (bf16 bias)
EXP_PATTERN = ("s", "v", "s", "v", "s")  # 3/5 ScalarE exact
BF16 = ml_dtypes.bfloat16

_COMPILED = {}


def build_nc():
    nc = bacc.Bacc(
        "TRN2",
        target_bir_lowering=False,
        debug=False,
        enable_asserts=True,
        num_devices=N_CORES,
    )
    f32 = mybir.dt.float32
    bf16 = mybir.dt.bfloat16
    i16 = mybir.dt.int16

    qt_d = nc.dram_tensor("qt", [DUOS * 2 * D, S], bf16, kind="ExternalInput").ap()
    kt_d = nc.dram_tensor("kt", [DUOS * 2 * D, S], bf16, kind="ExternalInput").ap()
    v_d = nc.dram_tensor("v", [PAIRS * S, D], bf16, kind="ExternalInput").ap()
    out_d = nc.dram_tensor("out", [PAIRS * S, D], f32, kind="ExternalOutput").ap()

    with tile.TileContext(nc) as tc:
        with (
            tc.tile_pool(name="consts", bufs=1) as consts,
            tc.tile_pool(name="qk", bufs=2) as qk_pool,
            tc.tile_pool(name="vp", bufs=2) as v_pool,
            tc.tile_pool(name="pp", bufs=8) as p_pool,
            tc.tile_pool(name="op", bufs=3) as o_pool,
            tc.tile_pool(name="fp", bufs=3) as f_pool,
            tc.tile_pool(name="rp", bufs=3) as r_pool,
            tc.tile_pool(name="big", bufs=3, space="PSUM") as big_pool,
            tc.tile_pool(name="acc", bufs=2, space="PSUM") as acc_pool,
        ):
            ident = consts.tile([D + 1, D + 1], bf16)
            make_identity(nc, ident)
            # 0/1 causal triangle for DVE-side diagonal masking
            trimask = consts.tile([KT, 2, KT], bf16)
            nc.gpsimd.memset(trimask, 1.0)
            nc.gpsimd.affine_select(
                out=trimask, in_=trimask,
                compare_op=mybir.AluOpType.is_ge, fill=0.0, base=0,
                pattern=[[0, 2], [1, KT]], channel_multiplier=-1,
            )
            st = {"exp": 0, "tail": 0}

            def load_duo(dd):
                qsb = qk_pool.tile([2 * D, S], bf16, tag="qsb", name=f"q{dd}")
                ksb = qk_pool.tile([2 * D, S], bf16, tag="ksb", name=f"k{dd}")
                r0, r1 = dd * 128, (dd + 1) * 128
                if dd == 0:
                    # first duo: land the j=0 columns first so the PE starts;
                    # the k chunk rides the idle vector queue at startup
                    nc.sync.dma_start(out=qsb[:, 0:QT], in_=qt_d[r0:r1, 0:QT])
                    nc.scalar.dma_start(out=ksb[:, 0:QT],
                                        in_=kt_d[r0:r1, 0:QT])
                    nc.sync.dma_start(out=qsb[:, QT:], in_=qt_d[r0:r1, QT:])
                    nc.sync.dma_start(out=ksb[:, QT:], in_=kt_d[r0:r1, QT:])
                else:
                    nc.sync.dma_start(out=qsb, in_=qt_d[r0:r1, :])
                    nc.sync.dma_start(out=ksb, in_=kt_d[r0:r1, :])
                vs = []
                for s_ in range(2):
                    h = 2 * dd + s_
                    vt = v_pool.tile([KT, S // KT, D + 1], bf16, tag=f"v{s_}",
                                     name=f"v{dd}_{s_}")
                    nc.gpsimd.memset(vt[:, :, D:D + 1], 1.0)
                    nc.sync.dma_start(
                        out=vt[:, :, 0:D],
                        in_=v_d[h * S:(h + 1) * S, :].rearrange(
                            "(t kp) d -> kp t d", kp=KT),
                    )
                    vs.append(vt)
                return qsb, ksb, vs

            def emit_s(sb, j, t):
                qsb, ksb, _ = sb
                off = max(0, KT * t - QT * j)
                w = QT - off
                q0 = QT * j + off
                ps = big_pool.tile([KT, 2, QT], f32, tag="ps", name="ps")
                nc.tensor.matmul(
                    ps[:, 0, 0:w],
                    lhsT=ksb[0:D, KT * t:KT * (t + 1)],
                    rhs=qsb[0:D, q0:QT * (j + 1)],
                    start=True, stop=True, tile_position=(0, 0),
                )
                nc.tensor.matmul(
                    ps[:, 1, 0:w],
                    lhsT=ksb[D:2 * D, KT * t:KT * (t + 1)],
                    rhs=qsb[D:2 * D, q0:QT * (j + 1)],
                    start=True, stop=True, tile_position=(64, 0),
                )
                p3 = p_pool.tile([KT, 2, QT], bf16, tag="p3", name="p3")
                eng = EXP_PATTERN[st["exp"] % len(EXP_PATTERN)]
                st["exp"] += 1
                if eng == "s":
                    nc.scalar.activation(
                        out=p3[:, :, 0:w], in_=ps[:, :, 0:w],
                        func=mybir.ActivationFunctionType.Exp, scale=SCALE,
                    )
                else:
                    nc.vector.tensor_scalar(
                        out=p3[:, :, 0:w].bitcast(i16), in0=ps[:, :, 0:w],
                        scalar1=A_SCH, scalar2=B_SCH,
                        op0=mybir.AluOpType.mult, op1=mybir.AluOpType.add,
                    )
                if t >= (QT // KT) * j:  # diagonal tile: zero q_rel < k_rel
                    if eng == "v":
                        # same engine as the exp: no cross-engine hop, and
                        # it halves the gpsimd mask serialization
                        nc.vector.tensor_tensor(
                            out=p3[:, :, 0:KT], in0=p3[:, :, 0:KT],
                            in1=trimask, op=mybir.AluOpType.mult,
                        )
                    else:
                        nc.gpsimd.affine_select(
                            out=p3[:, :, 0:KT], in_=p3[:, :, 0:KT],
                            compare_op=mybir.AluOpType.is_ge,
                            fill=0.0, base=0,
                            pattern=[[0, 2], [1, KT]], channel_multiplier=-1,
                        )
                return p3, off, w

            def emit_pv(sb, accs, t, p3off, first, last):
                p3, off, w = p3off
                for s_ in range(2):
                    nc.tensor.matmul(
                        accs[s_][:, off:QT],
                        lhsT=sb[2][s_][:, t, :],
                        rhs=p3[:, s_, 0:w],
                        start=first, stop=last,
                    )

            def emit_tail(h, j, acc):
                osb = o_pool.tile([D + 1, QT], bf16, tag="osb", name="osb")
                if st["tail"] % 2 == 0:
                    nc.vector.tensor_copy(osb, acc)
                else:
                    nc.scalar.copy(out=osb, in_=acc)
                st["tail"] += 1
                # transpose scratch carved from a ps slot's first bank
                pst = big_pool.tile([KT, 2, QT], f32, tag="ps", name="tpslot")
                tp = pst[:, 0, 0:(QT // KT) * (D + 1)].rearrange(
                    "p (b c) -> p b c", b=QT // KT)
                for b_ in range(QT // KT):
                    nc.tensor.matmul(
                        tp[:, b_, :],
                        lhsT=osb[:, KT * b_:KT * (b_ + 1)],
                        rhs=ident, start=True, stop=True,
                    )
                rinv = r_pool.tile([KT, QT // KT], f32, tag="ri", name="rinv")
                nc.vector.reciprocal(rinv, tp[:, :, D])
                fsb = f_pool.tile([KT, QT // KT, D], f32, tag="f", name="fsb")
                # one mul for all 4 blocks: rinv broadcast along d (stride 0)
                nc.vector.tensor_tensor(
                    out=fsb, in0=tp[:, :, 0:D],
                    in1=rinv.broadcast_to([KT, QT // KT, D]),
                    op=mybir.AluOpType.mult,
                )
                row0 = h * S + QT * j
                nc.sync.dma_start(
                    out=out_d[row0:row0 + QT, :].rearrange(
                        "(b p) d -> p b d", p=KT),
                    in_=fsb,
                )

            sbs = load_duo(0)
            sbs_next = None
            LAG = GR + 1
            pend = []           # (sb, accs, nkt, t, p3off, uid)
            npop = {}           # uid -> PVs emitted so far
            pending_tails = []  # (uid, head, j, acc)

            def pop_pv():
                psb, paccs, pnkt, tt, p3off, uid = pend.pop(0)
                k = npop.get(uid, 0)
                emit_pv(psb, paccs, tt, p3off,
                        first=(k == 0), last=(k == pnkt - 1))
                npop[uid] = k + 1

            def emit_ready_tails():
                # a unit's tails may go once all its PVs are emitted; placed
                # at group boundaries so the transpose matmuls never land
                # mid-PV-run (which breaks MM chaining)
                while pending_tails and npop.get(pending_tails[0][0], 0) == \
                        4 * (pending_tails[0][2] + 1):
                    emit_tail(*pending_tails.pop(0)[1:])

            for dd in range(DUOS):
                sb = sbs
                for j in range(NQT):
                    uid = dd * NQT + j
                    nkt = (QT // KT) * (j + 1)
                    accx = acc_pool.tile([D + 1, QT], f32, tag="acc",
                                         name="accx")
                    accy = acc_pool.tile([D + 1, QT], f32, tag="acc",
                                         name="accy")
                    accs = (accx, accy)
                    # spread diagonal k-tiles evenly (one per group):
                    # front-loading all 4 serializes their exp->mask chains
                    # past what the PE can cover, and they must stay off the
                    # end-of-unit flush
                    fi = iter(range(4 * j))
                    t_order = []
                    for dt in range(4 * j, nkt):
                        t_order.append(dt)
                        for _ in range(j):
                            t_order.append(next(fi))
                    for g in range(nkt // GR):
                        for half in range(GR):
                            t = t_order[GR * g + half]
                            pend.append((sb, accs, nkt, t,
                                         emit_s(sb, j, t), uid))
                        if g == 0 and j == 1 and dd + 1 < DUOS:
                            sbs_next = load_duo(dd + 1)
                        while len(pend) > LAG:
                            pop_pv()
                        emit_ready_tails()
                    pending_tails.append((uid, 2 * dd, j, accx))
                    pending_tails.append((uid, 2 * dd + 1, j, accy))
                sbs = sbs_next
            while pend:
                pop_pv()
            for args in pending_tails:
                emit_tail(*args[1:])

    nc.compile()
    return nc


def _get_nc():
    if "nc" not in _COMPILED:
        _COMPILED["nc"] = build_nc()
    return _COMPILED["nc"]


def make_in_maps(q, k, v):
    q = np.asarray(q, dtype=np.float32).reshape(B * H, S, D)
    k = np.asarray(k, dtype=np.float32).reshape(B * H, S, D)
    v = np.asarray(v, dtype=np.float32).reshape(B * H, S, D)
    in_maps = []
    for c in range(N_CORES):
        sl = slice(c * PAIRS, (c + 1) * PAIRS)
        # duo-stacked d-major [DUOS*128, S]: duo dd rows 0-63 = head 2dd,
        # rows 64-127 = head 2dd+1
        qt = np.ascontiguousarray(q[sl].transpose(0, 2, 1)).reshape(
            DUOS * 2 * D, S)
        kt = np.ascontiguousarray(k[sl].transpose(0, 2, 1)).reshape(
            DUOS * 2 * D, S)
        in_maps.append({
            "qt": qt.astype(BF16),
            "kt": kt.astype(BF16),
            "v": np.ascontiguousarray(v[sl]).reshape(PAIRS * S, D).astype(BF16),
        })
    return in_maps


def assemble(results):
    out = np.empty((B * H, S, D), dtype=np.float32)
    for c in range(N_CORES):
        out[c * PAIRS:(c + 1) * PAIRS] = results[c]["out"].reshape(PAIRS, S, D)
    return np.ascontiguousarray(
        out.reshape(B, H, S, D).transpose(0, 2, 1, 3).reshape(B, S, H * D))


def kernel(q, k, v):
    nc = _get_nc()
    res = bass_utils.run_bass_kernel_spmd(
        nc, make_in_maps(q, k, v), core_ids=list(range(N_CORES)))
    return assemble(res.results)
